# revision 17
# baseline (speedup 1.0000x reference)
"""GCN link-prediction kernel for Trainium2 (8 NeuronCores, Bass/Tile).

Math: gcn(x) = D^-1/2 (A+I) D^-1/2 (x W) + b.  With u = dinv * (x W) the
edge aggregation becomes a pure segment-sum s[i] = sum_{e: dst=i} u[src_e],
done as one-hot matmuls on TensorE (P^T @ gathered_rows accumulated in PSUM),
with edges sharded across cores by destination node.

Pipeline (all cores run one SPMD program; per-core behavior differs only
through input data):
  P1   : every core computes the full u1 = dinv*(x@W1) (bf16) -> own DRAM
  L1   : dst-sharded aggregation via dma_gather + one-hot matmuls -> h1
  L2a  : h1 -> u2 slice (PE transpose + W2 matmul + dinv scale)
  AG1  : AllGather u2 slices (bf16) + repad rows to 256B
  L2   : aggregation again (same edge tables) -> h2
  AG2  : AllGather h2 (bf16) + repad
  SC   : scoring-edge gathers + fused mul-reduce dot products -> scores
"""

import sys

sys.path.insert(0, "/opt/trn_rl_repo")

import numpy as np
import ml_dtypes

import concourse.bacc as bacc
import concourse.bass as bass
import concourse.mybir as mybir
import concourse.tile as tile
from concourse import library_config
from concourse.bass_utils import run_bass_kernel_spmd

# ----- problem constants (hardcoded; kernel.py must be self-contained) -----
N = 50000
F = 128          # input features == hidden1
H2 = 64          # hidden2
NCORE = 8
NPAD = 50176     # = 8 * 49 * 128
NLOC = NPAD // NCORE        # 6272 nodes per core
NWIN = NLOC // 128          # 49 local windows
GWIN = NPAD // 128          # 392 global windows
HALF = 32768                # int16 gather-index limit
GW = 7                      # windows per gather group (49 = 7*7)
NGRP = NWIN // GW
SC_SG = 64                  # scoring chunks per gather call

BF16 = mybir.dt.bfloat16
F32 = mybir.dt.float32
I16 = mybir.dt.int16

_cache = {}


# --------------------------------------------------------------------------
# Bass program
# --------------------------------------------------------------------------
def build_program(CLO, CHI, CS, score_regions, has_b1, has_b2):
    """CLO/CHI: lo/hi chunks per aggregation window. CS: total scoring chunks.
    score_regions: list of (col0, ncols, src_hi, dst_hi) per region."""
    import os
    PHASES = int(os.environ.get("KERNEL_PHASES", "7"))  # 1..7 cumulative
    GCH = GW * (CLO + CHI)          # chunks per gather group
    NCHUNK = NWIN * (CLO + CHI)     # edge chunks per core

    nc = bacc.Bacc("TRN2", target_bir_lowering=False, debug=False,
                   num_devices=NCORE)

    # ---- I/O ----
    xT_d = nc.dram_tensor("xT", [128, NPAD], BF16, kind="ExternalInput")
    xTl_d = nc.dram_tensor("xTloc", [128, NLOC], BF16, kind="ExternalInput")
    W1_d = nc.dram_tensor("W1", [128, F], BF16, kind="ExternalInput")
    W2_d = nc.dram_tensor("W2", [128, H2], BF16, kind="ExternalInput")
    b1_d = nc.dram_tensor("b1bc", [128, F], F32, kind="ExternalInput")
    b2_d = nc.dram_tensor("b2bc", [128, H2], F32, kind="ExternalInput")
    dg_d = nc.dram_tensor("dinvg", [128, GWIN], F32, kind="ExternalInput")
    dl_d = nc.dram_tensor("dinvloc", [128, NWIN], F32, kind="ExternalInput")
    io_d = nc.dram_tensor("iota", [128, 128], F32, kind="ExternalInput")
    id_d = nc.dram_tensor("ident", [128, 128], F32, kind="ExternalInput")
    lc_d = nc.dram_tensor("lc", [128, NCHUNK], F32, kind="ExternalInput")
    ei_d = nc.dram_tensor("eidx", [128, NCHUNK * 8], I16, kind="ExternalInput")
    ss_d = nc.dram_tensor("sidxs", [128, CS * 8], I16, kind="ExternalInput")
    sd_d = nc.dram_tensor("sidxd", [128, CS * 8], I16, kind="ExternalInput")
    out_d = nc.dram_tensor("scores", [128, CS], F32, kind="ExternalOutput")

    # ---- internal DRAM ----
    u1_d = nc.dram_tensor("u1", [NPAD, F], BF16)
    u2s_d = nc.dram_tensor("u2slice", [NLOC, H2], BF16)
    u2f_d = nc.dram_tensor("u2full", [NPAD, H2], BF16, addr_space="Shared")
    u2p_d = nc.dram_tensor("u2pad", [NPAD, 128], BF16)
    h2s_d = nc.dram_tensor("h2slice", [NLOC, H2], BF16)
    h2f_d = nc.dram_tensor("h2full", [NPAD, H2], BF16, addr_space="Shared")
    h2p_d = nc.dram_tensor("h2pad", [NPAD, 128], BF16)

    u1v = u1_d.rearrange("(w p) f -> p w f", p=128)      # [128, GWIN, F]
    u2sv = u2s_d.rearrange("(w p) f -> p w f", p=128)    # [128, NWIN, H2]
    h2sv = h2s_d.rearrange("(w p) f -> p w f", p=128)

    rg = [list(range(NCORE))]

    with tile.TileContext(nc) as tc:
        with (
            tc.tile_pool(name="pconst", bufs=1) as pc,
            tc.tile_pool(name="ppsum", bufs=2, space="PSUM") as pp,
            tc.tile_pool(name="pwork", bufs=4) as pw,
        ):
            nc.gpsimd.load_library(library_config.mlp)

            # load constants
            W1_t = pc.tile([128, F], BF16)
            nc.sync.dma_start(W1_t[:], W1_d[:])
            W2_t = pc.tile([128, H2], BF16)
            nc.sync.dma_start(W2_t[:], W2_d[:])
            dg_t = pc.tile([128, GWIN], F32)
            nc.sync.dma_start(dg_t[:], dg_d[:])
            dl_t = pc.tile([128, NWIN], F32)
            nc.sync.dma_start(dl_t[:], dl_d[:])
            io_t = pc.tile([128, 128], F32)
            nc.sync.dma_start(io_t[:], io_d[:])
            id_t = pc.tile([128, 128], F32)
            nc.sync.dma_start(id_t[:], id_d[:])
            lc_t = pc.tile([128, NCHUNK], F32)
            nc.sync.dma_start(lc_t[:], lc_d[:])
            ei_t = pc.tile([128, NCHUNK * 8], I16)
            nc.sync.dma_start(ei_t[:], ei_d[:])
            ss_t = pc.tile([128, CS * 8], I16)
            nc.sync.dma_start(ss_t[:], ss_d[:])
            sd_t = pc.tile([128, CS * 8], I16)
            nc.sync.dma_start(sd_t[:], sd_d[:])
            if has_b1:
                b1_t = pc.tile([128, F], F32)
                nc.sync.dma_start(b1_t[:], b1_d[:])
            if has_b2:
                b2_t = pc.tile([128, H2], F32)
                nc.sync.dma_start(b2_t[:], b2_d[:])

            with tc.tile_pool(name="pstate", bufs=1) as ps:
                u1self = ps.tile([128, NWIN, F], F32)
                h1_t = ps.tile([128, NWIN, F], F32)
                u2self = ps.tile([128, NWIN, H2], F32)
                u2bf = ps.tile([128, NWIN, H2], BF16)
                h2bf = ps.tile([128, NWIN, H2], BF16)

                # ============ P1: u1 = dinv * (x @ W1), all nodes ============
                for wb in range(0, GWIN, GW):
                    nw = min(GW, GWIN - wb)
                    xg = pw.tile([128, GW * 128], BF16, tag="xg", bufs=3)
                    nc.sync.dma_start(xg[:, 0:nw * 128],
                                      xT_d[:, wb * 128:(wb + nw) * 128])
                    u1b = pw.tile([128, GW, F], BF16, tag="u1b")
                    for j in range(nw):
                        t1 = pp.tile([128, F], F32, tag="mm")
                        nc.tensor.matmul(
                            t1[:], xg[:, j * 128:(j + 1) * 128], W1_t[:],
                            start=True, stop=True)
                        nc.vector.tensor_scalar_mul(
                            u1b[:, j, :], t1[:], dg_t[:, wb + j:wb + j + 1])
                    nc.sync.dma_start(u1v[:, wb:wb + nw, :], u1b[:, 0:nw, :])

                # local pass: keep f32 u1 of own slice for self-loop term
                for jb in range(0, NWIN, GW):
                    nw = min(GW, NWIN - jb)
                    xg = pw.tile([128, GW * 128], BF16, tag="xg", bufs=3)
                    nc.sync.dma_start(xg[:, 0:nw * 128],
                                      xTl_d[:, jb * 128:(jb + nw) * 128])
                    for j in range(nw):
                        t1 = pp.tile([128, F], F32, tag="mm")
                        nc.tensor.matmul(
                            t1[:], xg[:, j * 128:(j + 1) * 128], W1_t[:],
                            start=True, stop=True)
                        nc.vector.tensor_scalar_mul(
                            u1self[:, jb + j, :], t1[:], dl_t[:, jb + j:jb + j + 1])

                # ============ aggregation helper ============
                def aggregate(src_view_lo, src_view_hi, fdim, self_t, dst_t,
                              dst_dtype_bf, b_t, relu):
                    """One GCN aggregation layer over the shared edge tables.
                    dst_t[:, w, :fdim] = post(dinv*(sum + self) [+b] [relu])."""
                    for g in range(NGRP):
                        gt = pw.tile([128, GCH, 128], BF16, tag="gath", bufs=2)
                        c8 = g * GCH * 8
                        nlo = GW * CLO * 128
                        nhi = GW * CHI * 128
                        nc.gpsimd.dma_gather(
                            gt[:, 0:GW * CLO, :], src_view_lo,
                            ei_t[:, c8:c8 + GW * CLO * 8],
                            nlo, nlo, 128, single_packet=False)
                        nc.gpsimd.dma_gather(
                            gt[:, GW * CLO:GCH, :], src_view_hi,
                            ei_t[:, c8 + GW * CLO * 8:(g + 1) * GCH * 8],
                            nhi, nhi, 128, single_packet=False)
                        for j in range(GW):
                            w = g * GW + j
                            cols = ([j * CLO + c for c in range(CLO)]
                                    + [GW * CLO + j * CHI + c for c in range(CHI)])
                            acc = pp.tile([128, fdim], F32, tag="acc")
                            for ci, col in enumerate(cols):
                                P = pw.tile([128, 128], BF16, tag="P")
                                nc.vector.tensor_scalar(
                                    P[:], io_t[:], lc_t[:, g * GCH + col:g * GCH + col + 1],
                                    None, op0=mybir.AluOpType.is_equal)
                                nc.tensor.matmul(
                                    acc[:], P[:], gt[:, col, 0:fdim],
                                    start=(ci == 0), stop=(ci == len(cols) - 1))
                            e1 = pw.tile([128, fdim], F32, tag="e1")
                            nc.vector.tensor_tensor(
                                e1[:], acc[:], self_t[:, w, 0:fdim],
                                op=mybir.AluOpType.add)
                            if b_t is not None:
                                # e1 = e1*dinv + b ; then (relu)
                                nc.vector.tensor_scalar_mul(
                                    e1[:], e1[:], dl_t[:, w:w + 1])
                                nc.vector.tensor_tensor(
                                    e1[:], e1[:], b_t[:],
                                    op=mybir.AluOpType.add)
                                if relu:
                                    nc.scalar.activation(
                                        dst_t[:, w, 0:fdim], e1[:],
                                        mybir.ActivationFunctionType.Relu)
                                else:
                                    nc.vector.tensor_copy(
                                        dst_t[:, w, 0:fdim], e1[:])
                            else:
                                if relu:
                                    nc.scalar.activation(
                                        dst_t[:, w, 0:fdim], e1[:],
                                        mybir.ActivationFunctionType.Relu,
                                        scale=dl_t[:, w:w + 1])
                                else:
                                    nc.vector.tensor_scalar_mul(
                                        dst_t[:, w, 0:fdim], e1[:],
                                        dl_t[:, w:w + 1])

                # ============ L1 aggregation -> h1 (f32, SBUF) ============
                if PHASES >= 2:
                    aggregate(u1_d[:], u1_d[HALF:NPAD, :], F, u1self, h1_t,
                              False, b1_t if has_b1 else None, relu=True)

                # ============ L2 local: u2 = dinv * (h1 @ W2) ============
                for j in range(NWIN if PHASES >= 3 else 0):
                    hT = pp.tile([128, 128], F32, tag="tp")
                    nc.tensor.transpose(hT[:], h1_t[:, j, :], id_t[:])
                    hTb = pw.tile([128, 128], BF16, tag="hTb")
                    nc.vector.tensor_copy(hTb[:], hT[:])
                    t2 = pp.tile([128, H2], F32, tag="mm2")
                    nc.tensor.matmul(t2[:], hTb[:], W2_t[:], start=True, stop=True)
                    nc.vector.tensor_scalar_mul(
                        u2self[:, j, :], t2[:], dl_t[:, j:j + 1])
                    nc.vector.tensor_copy(u2bf[:, j, :], u2self[:, j, :])
                if PHASES >= 3:
                    nc.sync.dma_start(u2sv[:], u2bf[:])

                # ============ AG1 + repad ============
                if PHASES >= 4:
                    nc.gpsimd.collective_compute(
                        "AllGather", mybir.AluOpType.bypass, replica_groups=rg,
                        ins=[u2s_d[:]], outs=[u2f_d[:]])
                    nc.sync.dma_start(u2p_d[:, 0:H2], u2f_d[:])

                # ============ L2 aggregation -> h2 (bf16) ============
                if PHASES >= 5:
                    aggregate(u2p_d[:], u2p_d[HALF:NPAD, :], H2, u2self, h2bf,
                              True, b2_t if has_b2 else None, relu=False)
                    nc.sync.dma_start(h2sv[:], h2bf[:])

                # ============ AG2 + repad ============
                if PHASES >= 6:
                    nc.gpsimd.collective_compute(
                        "AllGather", mybir.AluOpType.bypass, replica_groups=rg,
                        ins=[h2s_d[:]], outs=[h2f_d[:]])
                    nc.sync.dma_start(h2p_d[:, 0:H2], h2f_d[:])

            # ============ scoring ============
            with tc.tile_pool(name="pscore", bufs=2) as psc:
                sc_t = psc.tile([128, CS], F32, bufs=1, tag="scores")
                if PHASES < 7:
                    score_regions = []
                    nc.gpsimd.memset(sc_t[:], 0.0)
                for (c0, ncols, shi, dhi) in score_regions:
                    for cb in range(c0, c0 + ncols, SC_SG):
                        n = min(SC_SG, c0 + ncols - cb)
                        gs = psc.tile([128, SC_SG, 128], BF16, tag="gs")
                        gd = psc.tile([128, SC_SG, 128], BF16, tag="gd")
                        src_v = h2p_d[HALF:NPAD, :] if shi else h2p_d[:]
                        dst_v = h2p_d[HALF:NPAD, :] if dhi else h2p_d[:]
                        nc.gpsimd.dma_gather(
                            gs[:, 0:n, :], src_v, ss_t[:, cb * 8:(cb + n) * 8],
                            n * 128, n * 128, 128, single_packet=False)
                        nc.gpsimd.dma_gather(
                            gd[:, 0:n, :], dst_v, sd_t[:, cb * 8:(cb + n) * 8],
                            n * 128, n * 128, 128, single_packet=False)
                        prod = psc.tile([128, SC_SG, H2], F32, tag="prod")
                        nc.vector.tensor_tensor(
                            prod[:, 0:n, :], gs[:, 0:n, 0:H2], gd[:, 0:n, 0:H2],
                            op=mybir.AluOpType.mult)
                        nc.vector.tensor_reduce(
                            sc_t[:, cb:cb + n], prod[:, 0:n, :],
                            axis=mybir.AxisListType.X, op=mybir.AluOpType.add)
                nc.sync.dma_start(out_d[:], sc_t[:])

    nc.compile()
    return nc


# --------------------------------------------------------------------------
# Host-side preparation
# --------------------------------------------------------------------------
def _prep(inputs):
    x = np.asarray(inputs["x"], np.float32)
    tei = np.asarray(inputs["train_pos_edge_index"], np.int64)
    pos = np.asarray(inputs["pos_edge_index"], np.int64)
    neg = np.asarray(inputs["neg_edge_index"], np.int64)
    W1 = np.asarray(inputs["W1"], np.float32)
    W2 = np.asarray(inputs["W2"], np.float32)
    b1 = np.asarray(inputs["b1"], np.float32)
    b2 = np.asarray(inputs["b2"], np.float32)

    row, col = tei[0], tei[1]
    deg = np.bincount(col, minlength=NPAD).astype(np.float64) + 1.0
    dinv = (1.0 / np.sqrt(deg)).astype(np.float32)
    dinv[N:] = 0.0

    xpad = np.zeros((NPAD, F), np.float32)
    xpad[:N] = x
    xT = np.ascontiguousarray(xpad.T).astype(ml_dtypes.bfloat16)

    dinv_pm = dinv.reshape(GWIN, 128).T.copy()          # [128, GWIN]

    # ---- train-edge tables ----
    owner = col // NLOC
    half = (row >= HALF).astype(np.int64)
    loc = col - owner * NLOC
    win = loc // 128
    lcv = loc % 128

    # counts per (core, win, half) to derive caps
    key = ((owner * NWIN + win) * 2 + half)
    cnt = np.bincount(key, minlength=NCORE * NWIN * 2).reshape(NCORE, NWIN, 2)
    CLO = int(np.ceil(cnt[:, :, 0].max() / 128))
    CHI = int(np.ceil(cnt[:, :, 1].max() / 128))
    GCH = GW * (CLO + CHI)
    NCHUNK = NWIN * (CLO + CHI)

    eidx16 = np.zeros((NCORE, 16, NCHUNK * 8), np.int16)
    lct = np.full((NCORE, 128, NCHUNK), -1.0, np.float32)

    # order edges by (owner, win, half) then assign slots sequentially
    order = np.lexsort((half, win, owner))
    ro, wo, ho, lo = row[order], win[order], half[order], lcv[order]
    oo = owner[order]
    # slot index within each (core,win,half) run
    runkey = ((oo * NWIN + wo) * 2 + ho)
    slot = np.arange(len(order)) - np.concatenate(
        ([0], np.cumsum(np.bincount(runkey, minlength=NCORE * NWIN * 2))))[runkey]
    # tile chunk column within group + global lc column
    g = wo // GW
    j = wo % GW
    chunk_in = slot // 128
    lane = slot % 128
    tile_col = np.where(ho == 0, j * CLO + chunk_in,
                        GW * CLO + j * CHI + chunk_in)
    gcol = g * GCH + tile_col
    idxval = np.where(ho == 0, ro, ro - HALF).astype(np.int16)
    eidx16[oo, lane % 16, gcol * 8 + lane // 16] = idxval
    lct[oo, lane, gcol] = lo.astype(np.float32)
    eidx = np.tile(eidx16, (1, 8, 1))          # replicate for 8 Q7 cores

    # ---- scoring tables ----
    sall = np.concatenate([pos, neg], axis=1)
    src, dst = sall[0], sall[1]
    ESC = src.shape[0]
    per = ESC // NCORE
    score_core = np.minimum(np.arange(ESC) // per, NCORE - 1)
    reg = (src >= HALF).astype(np.int64) * 2 + (dst >= HALF).astype(np.int64)
    rkey = score_core * 4 + reg
    rcnt = np.bincount(rkey, minlength=NCORE * 4).reshape(NCORE, 4)
    CSr = np.ceil(rcnt.max(axis=0) / 128).astype(np.int64)
    CS = int(CSr.sum())
    rbase = np.concatenate(([0], np.cumsum(CSr)))[:4]

    sidxs16 = np.zeros((NCORE, 16, CS * 8), np.int16)
    sidxd16 = np.zeros((NCORE, 16, CS * 8), np.int16)
    sperm = np.full((NCORE, CS * 128), -1, np.int64)

    sorder = np.lexsort((reg, score_core))
    so_src, so_dst = src[sorder], dst[sorder]
    so_core, so_reg = score_core[sorder], reg[sorder]
    srunkey = so_core * 4 + so_reg
    sslot = np.arange(ESC) - np.concatenate(
        ([0], np.cumsum(np.bincount(srunkey, minlength=NCORE * 4))))[srunkey]
    schunk = rbase[so_reg] + sslot // 128
    slane = sslot % 128
    sv = np.where(so_reg // 2 == 0, so_src, so_src - HALF).astype(np.int16)
    dv = np.where(so_reg % 2 == 0, so_dst, so_dst - HALF).astype(np.int16)
    sidxs16[so_core, slane % 16, schunk * 8 + slane // 16] = sv
    sidxd16[so_core, slane % 16, schunk * 8 + slane // 16] = dv
    sperm[so_core, schunk * 128 + slane] = sorder
    sidxs = np.tile(sidxs16, (1, 8, 1))
    sidxd = np.tile(sidxd16, (1, 8, 1))

    score_regions = []
    for r in range(4):
        score_regions.append((int(rbase[r]), int(CSr[r]), r // 2, r % 2))

    iota = np.tile(np.arange(128, dtype=np.float32), (128, 1))
    ident = np.eye(128, dtype=np.float32)
    has_b1 = bool(np.any(b1))
    has_b2 = bool(np.any(b2))

    in_maps = []
    for k in range(NCORE):
        sl = slice(k * NLOC, (k + 1) * NLOC)
        in_maps.append({
            "xT": xT,
            "xTloc": np.ascontiguousarray(xpad[sl].T).astype(ml_dtypes.bfloat16),
            "W1": W1.astype(ml_dtypes.bfloat16),
            "W2": W2.astype(ml_dtypes.bfloat16),
            "b1bc": np.tile(b1, (128, 1)).astype(np.float32),
            "b2bc": np.tile(b2, (128, 1)).astype(np.float32),
            "dinvg": dinv_pm,
            "dinvloc": dinv[sl].reshape(NWIN, 128).T.copy(),
            "iota": iota,
            "ident": ident,
            "lc": lct[k],
            "eidx": eidx[k],
            "sidxs": sidxs[k],
            "sidxd": sidxd[k],
        })

    meta = dict(CLO=CLO, CHI=CHI, CS=CS, score_regions=tuple(score_regions),
                has_b1=has_b1, has_b2=has_b2, sperm=sperm, ESC=ESC)
    return in_maps, meta


def kernel(**inputs):
    in_maps, meta = _prep(inputs)
    key = (meta["CLO"], meta["CHI"], meta["CS"], meta["score_regions"],
           meta["has_b1"], meta["has_b2"])
    if key not in _cache:
        _cache[key] = build_program(meta["CLO"], meta["CHI"], meta["CS"],
                                    list(meta["score_regions"]),
                                    meta["has_b1"], meta["has_b2"])
    nc = _cache[key]

    last_err = None
    for _attempt in range(3):
        try:
            res = run_bass_kernel_spmd(nc, in_maps, list(range(NCORE)))
            break
        except Exception as e:  # transient device-unrecoverable flakes
            last_err = e
    else:
        raise last_err

    out = np.empty(meta["ESC"], np.float32)
    sperm = meta["sperm"]
    for k in range(NCORE):
        arr = res.results[k]["scores"]          # [128, CS]
        flat = arr.T.reshape(-1)                # slot c*128+p -> arr[p, c]
        valid = sperm[k] >= 0
        out[sperm[k][valid]] = flat[valid]
    return out
